# revision 1
# baseline (speedup 1.0000x reference)
"""GCN layer (2x GCNConv + BatchNorm + ReLU) on 8 Trainium2 NeuronCores.

Strategy:
  - Nodes (M = 819200) are dest-sharded across 8 cores (102400 each); each
    core owns the edges whose destination falls in its shard.
  - Both aggregations run on width-2 features: layer 1 aggregates x before
    applying W1 (A@(x W1) == (A@x) W1); layer 2 applies W2 before
    aggregating.  Symmetric gcn_norm is factored as
    out[c] = dinv[c] * sum_e xs[row_e]  with xs = x * dinv[row].
  - Per-edge gather of xs rows is done with per-partition indirect DMA
    (128 descriptors / instruction), throttled through a two-buffer
    staging pool.
  - Segment sum over dest-sorted edges: exclusive prefix scan (DVE
    tensor_tensor_scan) + boundary extraction via a log-shift
    copy-predicated cascade (masks precomputed on host from the CSR
    pointers), then adjacent difference.
  - BatchNorm statistics are AllReduce'd across cores; the xs/ys tables are
    AllGather'd so each core can gather any source row.
"""

import numpy as np

N, T, V = 64, 512, 25
L = 2 * V
M = N * T * V            # 819200 nodes
P = 128                  # SBUF partitions
NCORES = 8
MC = M // NCORES         # 102400 dests per core
PD = MC // P             # 800 dests per partition
HID = 20
BN_EPS = 1e-5
PRE = 832                # dummy prefix ahead of the scan array (>= PD)
GCHUNK = 8               # gather columns per staging tile

_runtime = {}


def _setup_runtime():
    if _runtime:
        return _runtime
    import concourse.bass as bass
    import concourse.tile as tile
    from concourse import mybir
    import bass_rust
    from concourse.vector_clock import ScopedClock, VectorClock

    # --- patch 1: split the tail drain's sem waits (walrus rejects >2/inst)
    def _split_drain_and_barrier(self, tick_clock, wait_clock):
        nc = self.nc
        gc = tick_clock.global_clock
        n = len(gc)
        for p in range(n):
            t = gc[p]
            if t > 0:
                vc = VectorClock([t if i == p else 0 for i in range(n)])
                carrier = nc.sync.nop()
                wait_clock.add_sem_waits(carrier.ins, ScopedClock({None: vc}))
        nc.sync.drain()
        nc.all_engine_barrier()
        assert self.sems is not None
        popped = nc._tile_sem_poison_stack.pop()
        assert popped is self._sem_poison
        nc.clear_and_free_semaphores(list(self.sems.allocated().values()))
        nc.all_engine_barrier()

    # --- patch 2: any scheduled instruction with >2 sem waits gets the
    # excess moved onto EventSemaphore carriers inserted before it.
    MAXW = 1

    def _split_waits_in_blocks(self, ordered_blocks):
        nc = self.nc
        for bb_name, insts in ordered_blocks.items():
            new_list = []
            for inst in insts:
                si = inst.sync_info
                waits = list(si.on_wait) if (si and si.on_wait) else []
                if len(waits) > MAXW:
                    keep = waits[:MAXW - 1]
                    excess = waits[MAXW - 1:]
                    for k in range(0, len(excess), MAXW):
                        chunk = excess[k:k + MAXW]
                        carrier = mybir.InstEventSemaphore(
                            name=f"WSPLIT-{nc.next_id()}", ins=[], outs=[])
                        carrier.engine = inst.engine
                        carrier.sync_info = mybir.SyncInfo(
                            on_wait=list(chunk), on_update=[])
                        carrier.debug = inst.debug
                        new_list.append(carrier)
                    inst.sync_info = mybir.SyncInfo(
                        on_wait=keep,
                        on_update=list(si.on_update) if si.on_update else [])
                new_list.append(inst)
            insts[:] = new_list

    _orig_lower = tile.TileContext._lower_ordered_insts

    def _patched_lower(self, postordered_blocks):
        _split_waits_in_blocks(self, postordered_blocks)
        return _orig_lower(self, postordered_blocks)

    tile.TileContext._drain_and_barrier = _split_drain_and_barrier
    if getattr(tile.TileContext._lower_ordered_insts, "__name__", "") != "_patched_lower":
        tile.TileContext._lower_ordered_insts = _patched_lower

    _runtime["bass"] = bass
    _runtime["tile"] = tile
    _runtime["mybir"] = mybir
    return _runtime


# --------------------------------------------------------------------------
# host-side preprocessing (index manipulation only)
# --------------------------------------------------------------------------

def _cascade_masks(lptr, ES):
    """Masks for the boundary-extraction cascade (butterfly concentrator).

    Per partition we hold A = [PRE dummy pairs | Pex (ES+1 pairs)] and want
    A[d] -> A[g(d)] with g(d) = PRE + lptr[p, d] (non-decreasing).  Stages
    execute shift s ASCENDING:  A'[x] = m_s[x] ? A[x + s] : A[x]  over the
    full array (snapshot semantics).  The address-resolution chain therefore
    applies the LARGEST shift first; masks are built by simulating the chain
    MSB-first, scattering each route's bit to its current position.  Routing
    a non-decreasing map this way is conflict-free; conflicts are asserted.
    Returns shifts (ascending) and full-width masks [P, WA] per shift.
    """
    W = PRE                       # number of routed outputs (>= PD+1)
    WA = PRE + ES + 1             # full array width (pairs)
    nparts, npd1 = lptr.shape     # [128, PD+1]
    g = np.empty((nparts, W), np.int64)
    g[:, :npd1] = PRE + lptr
    # keep g strictly increasing over the padded tail (don't-care reads)
    g[:, npd1:] = (PRE + lptr[:, -1:]) + np.arange(1, W - npd1 + 1)[None, :]
    d = np.arange(W)[None, :]
    o = g - d
    assert (o >= 0).all() and int(g.max()) < WA
    nbits = max(1, int(np.ceil(np.log2(int(o.max()) + 1))))
    pos = np.broadcast_to(d, (nparts, W)).copy()
    rowoff = (np.arange(nparts) * WA)[:, None]
    masks_by_shift = {}
    for j in range(nbits - 1, -1, -1):      # chain order: MSB first
        b = ((o >> j) & 1).astype(np.uint8)
        lo = np.full(nparts * WA, 2, np.int8)
        hi = np.full(nparts * WA, -1, np.int8)
        flat = (rowoff + pos).ravel()
        np.minimum.at(lo, flat, b.ravel().astype(np.int8))
        np.maximum.at(hi, flat, b.ravel().astype(np.int8))
        used = hi >= 0
        assert (lo[used] == hi[used]).all(), "cascade routing conflict"
        m = np.zeros(nparts * WA, np.uint8)
        m[used] = hi[used].astype(np.uint8)
        masks_by_shift[1 << j] = m.reshape(nparts, WA)
        pos = pos + (b.astype(np.int64) << j)
    assert (pos == g).all()
    shifts = sorted(masks_by_shift)         # ascending execution order
    masks = [masks_by_shift[s] for s in shifts]
    return shifts, masks


def _host_prep(edge_index):
    row = np.asarray(edge_index[0], dtype=np.int64)
    col = np.asarray(edge_index[1], dtype=np.int64)
    deg = np.bincount(col, minlength=M).astype(np.float32) + 1.0

    per_core = []
    ES = 0
    for k in range(NCORES):
        sel = (col >= k * MC) & (col < (k + 1) * MC)
        r = row[sel].astype(np.int64)
        c = (col[sel] - k * MC).astype(np.int64)
        # dummy slot for every zero-degree dest (keeps the boundary map
        # strictly increasing => conflict-free cascade).  Dummy rows point
        # at this core's zero row in the (MC+1)-blocked shared table.
        dcnt = np.bincount(c, minlength=MC)
        zdest = np.nonzero(dcnt == 0)[0]
        zrow = np.full(zdest.shape, k * MC, np.int64)  # placeholder, remapped
        r = np.concatenate([r, zrow])
        zmark = np.concatenate([np.zeros(len(c), bool), np.ones(len(zdest), bool)])
        c = np.concatenate([c, zdest])
        order = np.argsort(c, kind="stable")
        r, c, zmark = r[order], c[order], zmark[order]
        # remap global row id -> blocked table id (core j's row r at j*(MC+1)+r%MC)
        widx_val = r + r // MC
        widx_val[zmark] = k * (MC + 1) + MC       # zero row of this core
        part = c // PD
        cnt = np.bincount(part, minlength=P)
        ES = max(ES, int(cnt.max()))
        per_core.append((widx_val.astype(np.int32), c, part, cnt))
    ES = int(np.ceil((ES + 40) / GCHUNK) * GCHUNK)

    cores = []
    for k in range(NCORES):
        r, c, part, cnt = per_core[k]
        widx = np.zeros((P, ES), np.int32)
        lptr = np.zeros((P, PD + 1), np.int64)
        starts = np.concatenate([[0], np.cumsum(cnt)])
        for p in range(P):
            sl = slice(starts[p], starts[p + 1])
            widx[p, :cnt[p]] = r[sl]
            # cumulative count per local dest within the partition
            loc = c[sl] - p * PD
            lptr[p] = np.searchsorted(loc, np.arange(PD + 1))
        shifts, masks = _cascade_masks(lptr, ES)
        degf = deg[k * MC:(k + 1) * MC].reshape(P, PD)
        cores.append(dict(widx=widx, shifts=shifts,
                          masks=masks, degf=degf))
    # unify cascade shift count across cores (SPMD: same program)
    all_shifts = sorted({s for cd in cores for s in cd["shifts"]})
    WA = PRE + ES + 1
    for cd in cores:
        sh2m = dict(zip(cd["shifts"], cd["masks"]))
        zero = np.zeros((P, WA), np.uint8)
        cd["masks"] = [sh2m.get(s, zero) for s in all_shifts]
        cd["shifts"] = all_shifts
    return ES, all_shifts, cores, deg


# --------------------------------------------------------------------------
# device program
# --------------------------------------------------------------------------

def _build_program(ES, shifts):
    rt = _setup_runtime()
    bass, tile, mybir = rt["bass"], rt["tile"], rt["mybir"]
    f32, i32, u8 = mybir.dt.float32, mybir.dt.int32, mybir.dt.uint8
    AF = mybir.ActivationFunctionType
    ALU = mybir.AluOpType
    nc = bass.Bass(target_bir_lowering=False)

    WA = PRE + ES + 1
    xloc = nc.declare_dram_parameter("xloc", [P, PD, 2], f32, isOutput=False)
    degf = nc.declare_dram_parameter("degf", [P, PD], f32, isOutput=False)
    widx = nc.declare_dram_parameter("widx", [P, ES], i32, isOutput=False)
    bmasks = nc.declare_dram_parameter("bmasks", [len(shifts), P, WA], u8,
                                       isOutput=False)
    w1 = nc.declare_dram_parameter("w1", [2, HID], f32, isOutput=False)
    gamma = nc.declare_dram_parameter("gamma", [1, HID], f32, isOutput=False)
    beta = nc.declare_dram_parameter("beta", [1, HID], f32, isOutput=False)
    w2 = nc.declare_dram_parameter("w2", [HID, 2], f32, isOutput=False)
    b2 = nc.declare_dram_parameter("b2", [1, 2], f32, isOutput=False)
    out_ext = nc.declare_dram_parameter("out", [P, PD, 2], f32, isOutput=True)

    # internal DRAM; table is blocked per core as (MC+1) rows, last row zero
    shard = nc.dram_tensor("shard", [(MC + 1) * 2], f32)
    table = nc.dram_tensor("table", [M + NCORES, 2], f32, addr_space="Shared")
    bn_in = nc.dram_tensor("bn_in", [2 * HID], f32)
    bn_out = nc.dram_tensor("bn_out", [2 * HID], f32, addr_space="Shared")
    groups = [list(range(NCORES))]

    from concourse.masks import make_identity

    with tile.TileContext(nc) as tc:
        with (
            tc.tile_pool(name="big", bufs=1) as big,
            tc.tile_pool(name="gst", bufs=3) as gst,
            tc.tile_pool(name="small", bufs=1) as small,
            tc.tile_pool(name="ps", bufs=2, space="PSUM") as psp,
        ):
            # ---- constants / inputs ----
            widx_t = big.tile([P, ES], i32)
            nc.sync.dma_start(out=widx_t[:], in_=widx[:])
            xl = big.tile([P, PD, 2], f32)
            nc.sync.dma_start(out=xl[:], in_=xloc[:])
            dg = big.tile([P, PD], f32)
            nc.sync.dma_start(out=dg[:], in_=degf[:])
            def part_bcast(ap):
                return bass.AP(tensor=ap.tensor, offset=ap.offset,
                               ap=[[0, P], *ap.ap])

            w1_t = small.tile([P, 2 * HID], f32)   # col-major [f*HID + j]
            nc.sync.dma_start(out=w1_t[:], in_=part_bcast(w1[:, :]))
            w2_t = small.tile([P, HID * 2], f32)   # [j*2 + f]
            nc.sync.dma_start(out=w2_t[:], in_=part_bcast(w2[:, :]))
            gm_t = small.tile([P, HID], f32)
            nc.sync.dma_start(out=gm_t[:], in_=part_bcast(gamma[0, :]))
            bt_t = small.tile([P, HID], f32)
            nc.sync.dma_start(out=bt_t[:], in_=part_bcast(beta[0, :]))
            b2_t = small.tile([P, 2], f32)
            nc.sync.dma_start(out=b2_t[:], in_=part_bcast(b2[0, :]))
            ident = small.tile([P, P], f32)
            make_identity(nc, ident)

            # dinv = 1/sqrt(deg)
            dinv = big.tile([P, PD], f32)
            nc.scalar.activation(out=dinv[:], in_=dg[:], func=AF.Sqrt)
            nc.vector.reciprocal(out=dinv[:], in_=dinv[:])

            def bcast_pd2(t):  # [P, PD] -> [P, PD, 2] broadcast AP
                a = t[:]
                return bass.AP(tensor=a.tensor, offset=a.offset,
                               ap=[a.ap[0], a.ap[1], [0, 2]])

            def mul_dinv(dst, src):
                nc.vector.tensor_tensor(out=dst[:], in0=src[:],
                                        in1=bcast_pd2(dinv), op=ALU.mult)

            # ---- per layer machinery ----
            msg = big.tile([P, ES, 2], f32)
            A = big.tile([P, PRE + ES + 1, 2], f32)
            agg = big.tile([P, PD, 2], f32)
            zero1 = small.tile([P, 2], f32)
            nc.vector.memset(zero1[:], 0.0)

            def gather_layer():
                ncols = ES // GCHUNK
                for cchunk in range(ncols):
                    stg = gst.tile([P, GCHUNK, 2], f32, tag="stg")
                    for u in range(GCHUNK):
                        i = cchunk * GCHUNK + u
                        nc.gpsimd.indirect_dma_start(
                            out=stg[:, u, :], out_offset=None, in_=table[:],
                            in_offset=bass.IndirectOffsetOnAxis(
                                ap=widx_t[:, i:i + 1], axis=0),
                        )
                    nc.vector.tensor_copy(
                        out=msg[:, cchunk * GCHUNK:(cchunk + 1) * GCHUNK, :],
                        in_=stg[:, :, :])

            def aggregate(own):
                """agg = segment-sum(msg) + own ; then *dinv."""
                # exclusive prefix scan of each feature into A[PRE:]
                nc.vector.memset(A[:, :PRE + 1, :], 0.0)
                for f in range(2):
                    ma = msg[:]
                    src = bass.AP(tensor=ma.tensor, offset=ma.offset + f,
                                  ap=[ma.ap[0], [2, ES]])
                    aa = A[:]
                    dst = bass.AP(tensor=aa.tensor,
                                  offset=aa.offset + (PRE + 1) * 2 + f,
                                  ap=[aa.ap[0], [2, ES]])
                    zb = bass.AP(tensor=zero1.tensor, offset=zero1[:].offset,
                                 ap=[zero1[:].ap[0], [0, ES]])
                    nc.vector.tensor_tensor_scan(
                        out=dst, data0=src, data1=zb, initial=0.0,
                        op0=ALU.add, op1=ALU.add)
                # cascade: A[x] = m ? A[x+s] : A[x], snapshot semantics,
                # shifts ascending, full width, masks streamed from DRAM
                for si, s in enumerate(shifts):
                    wdt = WA - s
                    mt = gst.tile([P, WA], u8, tag="cmask")
                    nc.sync.dma_start(out=mt[:], in_=bmasks[si])
                    mm = mt[:, :wdt]
                    mba = bass.AP(tensor=mm.tensor, offset=mm.offset,
                                  ap=[mm.ap[0], mm.ap[1], [0, 2]])
                    nc.vector.copy_predicated(
                        out=A[:, 0:wdt, :], mask=mba, data=A[:, s:s + wdt, :])
                # agg = diff of consecutive boundaries + own
                nc.vector.tensor_tensor(out=agg[:], in0=A[:, 1:PD + 1, :],
                                        in1=A[:, 0:PD, :], op=ALU.subtract)
                nc.vector.tensor_tensor(out=agg[:], in0=agg[:], in1=own[:],
                                        op=ALU.add)
                mul_dinv(agg, agg)

            def publish(src):
                """src [P, PD, 2] -> own rows of the shared table."""
                nc.sync.dma_start(out=shard[:MC * 2], in_=src[:])
                nc.sync.dma_start(out=shard[MC * 2:], in_=zero1[:1, :])
                cc = nc.gpsimd.collective_compute(
                    "AllGather", ALU.bypass, replica_groups=groups,
                    ins=[shard[:]], outs=[table[:]],
                )
                return cc

            # =========== layer 1 ===========
            xs = big.tile([P, PD, 2], f32)
            mul_dinv(xs, xl)
            publish(xs)
            gather_layer()
            aggregate(xs)

            # h[:, j, :] = agg0*w1[0,j] + agg1*w1[1,j]  (channel-major)
            h = big.tile([P, HID, PD], f32)
            ag = agg[:]
            a0 = bass.AP(tensor=ag.tensor, offset=ag.offset, ap=[ag.ap[0], [2, PD]])
            a1 = bass.AP(tensor=ag.tensor, offset=ag.offset + 1, ap=[ag.ap[0], [2, PD]])
            for j in range(HID):
                nc.scalar.activation(out=h[:, j, :], in_=a0, func=AF.Copy,
                                     scale=w1_t[:, j:j + 1])
                nc.vector.scalar_tensor_tensor(
                    out=h[:, j, :], in0=a1, scalar=w1_t[:, HID + j:HID + j + 1],
                    in1=h[:, j, :], op0=ALU.mult, op1=ALU.add)

            # ---- batch norm stats ----
            st = small.tile([P, 2 * HID], f32)
            nc.vector.tensor_reduce(out=st[:, :HID], in_=h[:],
                                    axis=mybir.AxisListType.X, op=ALU.add)
            sqscratch = small.tile([P, PD], f32)
            for j in range(HID):
                nc.scalar.activation(
                    out=sqscratch[:], in_=h[:, j, :], func=AF.Square,
                    accum_out=st[:, HID + j:HID + j + 1])
            # cross-partition reduce via ones-matmul: [1,P] @ [P,40] -> [1,40]
            ones = small.tile([P, 1], f32)
            nc.vector.memset(ones[:], 1.0)
            stp = psp.tile([P, 2 * HID], f32, space="PSUM")
            nc.tensor.matmul(out=stp[:1, :], lhsT=ones[:], rhs=st[:],
                             start=True, stop=True)
            sred = small.tile([1, 2 * HID], f32)
            nc.vector.tensor_copy(out=sred[:], in_=stp[:1, :])
            nc.sync.dma_start(out=bn_in[:], in_=sred[:])
            nc.gpsimd.collective_compute(
                "AllReduce", ALU.add, replica_groups=groups,
                ins=[bn_in[:]], outs=[bn_out[:]])
            sums = small.tile([P, 2 * HID], f32)
            nc.sync.dma_start(out=sums[:], in_=part_bcast(bn_out[:]))
            # mean, var, then scale s = gamma*rsqrt(var+eps), t = beta - mean*s
            mv = small.tile([P, 2 * HID], f32)   # [mean | var]
            nc.vector.tensor_scalar_mul(mv[:, :HID], sums[:, :HID], 1.0 / M)
            nc.vector.tensor_scalar_mul(mv[:, HID:], sums[:, HID:], 1.0 / M)
            nc.vector.tensor_tensor(out=sums[:, :HID], in0=mv[:, :HID],
                                    in1=mv[:, :HID], op=ALU.mult)
            nc.vector.tensor_tensor(out=mv[:, HID:], in0=mv[:, HID:],
                                    in1=sums[:, :HID], op=ALU.subtract)
            sbn = small.tile([P, 2 * HID], f32)  # [s | t]
            nc.vector.tensor_scalar_add(mv[:, HID:], mv[:, HID:], BN_EPS)
            nc.scalar.activation(out=sbn[:, :HID], in_=mv[:, HID:], func=AF.Sqrt)
            nc.vector.reciprocal(out=sbn[:, :HID], in_=sbn[:, :HID])
            nc.vector.tensor_tensor(out=sbn[:, :HID], in0=sbn[:, :HID],
                                    in1=gm_t[:], op=ALU.mult)
            nc.vector.tensor_tensor(out=sbn[:, HID:], in0=mv[:, :HID],
                                    in1=sbn[:, :HID], op=ALU.mult)
            nc.vector.tensor_tensor(out=sbn[:, HID:], in0=bt_t[:],
                                    in1=sbn[:, HID:], op=ALU.subtract)

            # h <- relu(h*s + t); then ys = (h @ W2) * dinv
            for j in range(HID):
                nc.scalar.activation(out=h[:, j, :], in_=h[:, j, :],
                                     func=AF.Relu,
                                     scale=sbn[:, j:j + 1],
                                     bias=sbn[:, HID + j:HID + j + 1])
            y2 = big.tile([P, PD, 2], f32)
            yv = y2[:]
            for f in range(2):
                yf = bass.AP(tensor=yv.tensor, offset=yv.offset + f,
                             ap=[yv.ap[0], [2, PD]])
                nc.scalar.activation(out=yf, in_=h[:, 0, :], func=AF.Copy,
                                     scale=w2_t[:, f:f + 1])
                for j in range(1, HID):
                    nc.vector.scalar_tensor_tensor(
                        out=yf, in0=h[:, j, :],
                        scalar=w2_t[:, j * 2 + f:j * 2 + f + 1],
                        in1=yf, op0=ALU.mult, op1=ALU.add)
            ys = big.tile([P, PD, 2], f32)
            mul_dinv(ys, y2)

            # =========== layer 2 ===========
            publish(ys)
            gather_layer()
            aggregate(ys)
            # out = agg + b2
            b2b = bass.AP(tensor=b2_t.tensor, offset=b2_t[:].offset,
                          ap=[b2_t[:].ap[0], [0, PD], [1, 2]])
            nc.vector.tensor_tensor(out=agg[:], in0=agg[:], in1=b2b, op=ALU.add)
            nc.sync.dma_start(out=out_ext[:], in_=agg[:])

    return nc


_prog_cache = {}


LAST_EXEC_NS = None


def _install_ntff_shim():
    """Provide antenv.axon_hooks if missing so trace=True works."""
    import sys as _sys
    import types, contextlib, ctypes
    if "antenv.axon_hooks" in _sys.modules:
        return
    try:
        import antenv.axon_hooks  # noqa: F401
        return
    except ImportError:
        pass
    so_path = "/opt/axon/libaxon_pjrt.so"

    def _make_hook():
        lib = ctypes.CDLL(so_path)
        if not hasattr(lib, "axon_start_nrt_profile"):
            return None
        lib.axon_start_nrt_profile.argtypes = [
            ctypes.POINTER(ctypes.c_int64), ctypes.c_size_t]
        lib.axon_start_nrt_profile.restype = ctypes.c_int64
        lib.axon_stop_nrt_profile.argtypes = [ctypes.c_char_p]
        lib.axon_stop_nrt_profile.restype = ctypes.c_int64

        @contextlib.contextmanager
        def _hook_cm(output_dir, device_ids):
            import jax
            jax.devices()
            if device_ids:
                ids = (ctypes.c_int64 * len(device_ids))(*device_ids)
                rc = lib.axon_start_nrt_profile(ids, len(device_ids))
            else:
                rc = lib.axon_start_nrt_profile(None, 0)
            if rc != 0:
                raise RuntimeError(f"axon_start_nrt_profile rc={rc}")
            try:
                yield
            finally:
                lib.axon_stop_nrt_profile(str(output_dir).encode())

        return _hook_cm

    hook = [None]

    def get_axon_ntff_profile_hook():
        if hook[0] is None:
            hook[0] = _make_hook()
        return hook[0]

    mod = types.ModuleType("antenv.axon_hooks")
    mod.get_axon_ntff_profile_hook = get_axon_ntff_profile_hook
    mod.set_axon_ntff_profile_hook = lambda h: hook.__setitem__(0, h)
    _sys.modules["antenv.axon_hooks"] = mod


def kernel(x, edge_index, W1, b1, gamma, beta, W2, b2):
    global LAST_EXEC_NS
    import os
    from concourse.bass_utils import run_bass_kernel_spmd

    x = np.asarray(x)
    xf = x.reshape(M, 2).astype(np.float32)
    ES, shifts, cores, deg = _host_prep(np.asarray(edge_index))

    key = (ES, tuple(shifts))
    if key not in _prog_cache:
        _prog_cache[key] = _build_program(ES, shifts)
    nc = _prog_cache[key]

    in_maps = []
    for k in range(NCORES):
        cd = cores[k]
        in_maps.append({
            "xloc": xf[k * MC:(k + 1) * MC].reshape(P, PD, 2),
            "degf": cd["degf"],
            "widx": cd["widx"],
            "bmasks": np.stack(cd["masks"]).astype(np.uint8),
            "w1": np.asarray(W1, np.float32),
            "gamma": np.asarray(gamma, np.float32).reshape(1, HID),
            "beta": np.asarray(beta, np.float32).reshape(1, HID),
            "w2": np.asarray(W2, np.float32),
            "b2": np.asarray(b2, np.float32).reshape(1, 2),
        })
    trace = os.environ.get("GCN_TRACE") == "1"
    if trace:
        _install_ntff_shim()
    res = None
    last_exc = None
    for attempt in range(3):
        try:
            res = run_bass_kernel_spmd(nc, in_maps, list(range(NCORES)),
                                       trace=trace)
            break
        except Exception as e:           # transient device errors: retry
            last_exc = e
            import time as _time
            _time.sleep(3.0)
    if res is None:
        raise last_exc
    if res.exec_time_ns is not None:
        LAST_EXEC_NS = res.exec_time_ns
    out = np.concatenate([res.results[k]["out"].reshape(MC, 2)
                          for k in range(NCORES)], axis=0)
    return out.reshape(N, T, L).astype(np.float32)

